# revision 18
# baseline (speedup 1.0000x reference)
"""Trainium2 Bass kernel for 3D deformable attention (8 NeuronCores).

Sharding: core c = n*2 + hg  (n = batch sample 0..3, hg = head-group 0/1,
4 heads each).  Each core computes a partial output (its 4 heads pushed
through W_out); the host sums core pairs and adds b_out.

Device algorithm per core (q = query 0..16383, qh = q//8192,
qol = (q%8192)//128, qi = q%128; h = local head 0..3; p = point 0..3):

  B) value projection -> V_T [128=(h,c), r] bf16 (column layout); 7
     finite-difference volumes G_abc (zero-extended per axis, a/b/c bits
     for Dd/Dy/Dx); PE-transpose to row layout; DRAM table per head:
     table[h][r][c*8+g] bf16 (512 B rows).
  C) offset/attn projection; pixel coords; exact floor/frac; per-axis
     (alpha, beta) coefs + clipped base c0; attn softmax — all in
     coordinate-major layout [row = (qh, axis, m, p), col = q];
     PE-transpose into sample-major layout [qi, grp, h, slot],
     slot = qol*8 + p*2 + qh.
  D) r0 = (c0d*32+c0y)*32+c0x; fold into the gather's wrapped index
     layout idxw[qi%16, qi//16 + 8*slot] via 8 permutation matmuls.
  E) per head: dma_gather(table[h], idxw) -> gbuf[qi, slot, (c,g)];
     multiply by 8-term polynomial coefs c8 (attn prefolded into the
     d-axis pair) and tensor_reduce over g; fold p ->
     sampled[qi, h, qol, qh, c] bf16.
  F) PE-transpose sampled to [(h,c), q] chunks; out-proj matmul with
     W_out slice; DMA partial [16384, 256] f32.
"""

import sys

sys.path.insert(0, "/opt/trn_rl_repo")

import numpy as np

import concourse.bass as bass
import concourse.mybir as mybir
import concourse.tile as tile
from concourse import bacc
from concourse.bass_utils import run_bass_kernel_spmd

F32 = mybir.dt.float32
BF16 = mybir.dt.bfloat16
I16 = mybir.dt.int16
I32 = mybir.dt.int32
AX = mybir.AxisListType
OP = mybir.AluOpType
ACTF = mybir.ActivationFunctionType

D_, H_, W_ = 16, 32, 32
LEN = D_ * H_ * W_          # 16384
DIM = 256
M_TOT, P = 8, 4
HM = 4                      # heads per core
DH = 32
QH = LEN // 2               # 8192
NBLK = 8
BLK = QH // NBLK            # 1024
RCH = 4096                  # r-chunk for G build
NSLOT = 512                 # slots per head = 64 qol * 4 p * 2 qh
GCALL = 4096                # gather idxs per call
GSLOT = GCALL // 128        # 32
NCALL = NSLOT // GSLOT      # 16


def build_program(niter=1):
    nc = bacc.Bacc("TRN2", target_bir_lowering=False, debug=False,
                   num_devices=8)

    xt = nc.declare_dram_parameter("xt", [2, 128, LEN], F32, isOutput=False)
    grid = nc.declare_dram_parameter("grid", [128, QH], F32, isOutput=False)
    wv = nc.declare_dram_parameter("wv", [2, 128, 128], BF16, isOutput=False)
    wproj = nc.declare_dram_parameter("wproj", [2, 128, 64], BF16, isOutput=False)
    wout = nc.declare_dram_parameter("wout", [128, 256], BF16, isOutput=False)
    bval = nc.declare_dram_parameter("bval", [128, 1], F32, isOutput=False)
    bproj = nc.declare_dram_parameter("bproj", [128, 1], F32, isOutput=False)
    selsum = nc.declare_dram_parameter("selsum", [128, 8], BF16, isOutput=False)
    selrep = nc.declare_dram_parameter("selrep", [8, 64], F32, isOutput=False)
    selvr = nc.declare_dram_parameter("selvr", [3, 128], F32, isOutput=False)
    selss = nc.declare_dram_parameter("selss", [3, 128], F32, isOutput=False)
    vr3 = nc.declare_dram_parameter("vr3", [3, 1], F32, isOutput=False)
    ss3 = nc.declare_dram_parameter("ss3", [3, 1], F32, isOutput=False)
    shr = nc.declare_dram_parameter("shr", [128, 1], F32, isOutput=False)
    shc = nc.declare_dram_parameter("shc", [128, 1], F32, isOutput=False)
    mxr = nc.declare_dram_parameter("mxr", [128, 1], F32, isOutput=False)
    pfold = nc.declare_dram_parameter("pfold", [128, 8, 16], F32, isOutput=False)
    idb = nc.declare_dram_parameter("idb", [128, 128], BF16, isOutput=False)
    outp = nc.declare_dram_parameter("outp", [LEN, 256], F32, isOutput=True)

    with tile.TileContext(nc) as tc:
        with (
            tc.tile_pool(name="const", bufs=1) as cpool,
            tc.tile_pool(name="dram", bufs=1, space="DRAM") as dpool,
            tc.tile_pool(name="coef", bufs=1) as fpool,
        ):
            # ---------- constants ----------
            def cload(src, shape, dtype, name):
                t = cpool.tile(shape, dtype, tag=name)
                nc.sync.dma_start(out=t[:], in_=src[:])
                return t

            wv_sb = cpool.tile([128, 2, 128], BF16, tag="wv")
            nc.sync.dma_start(out=wv_sb[:, 0, :], in_=wv[0])
            nc.sync.dma_start(out=wv_sb[:, 1, :], in_=wv[1])
            wp_sb = cpool.tile([128, 2, 64], BF16, tag="wp")
            nc.sync.dma_start(out=wp_sb[:, 0, :], in_=wproj[0])
            nc.sync.dma_start(out=wp_sb[:, 1, :], in_=wproj[1])
            pfold_sb = cload(pfold, [128, 8, 16], F32, "pfold")
            wo_sb = cload(wout, [128, 256], BF16, "wo")
            bval_sb = cload(bval, [128, 1], F32, "bval")
            bproj_sb = cload(bproj, [128, 1], F32, "bproj")
            selsum_sb = cload(selsum, [128, 8], BF16, "selsum")
            selrep_sb = cload(selrep, [8, 64], F32, "selrep")
            selvr_sb = cload(selvr, [3, 128], F32, "selvr")
            selss_sb = cload(selss, [3, 128], F32, "selss")
            vr_sb = cload(vr3, [3, 1], F32, "vr3")
            ss_sb = cload(ss3, [3, 1], F32, "ss3")
            shr_sb = cload(shr, [128, 1], F32, "shr")
            shc_sb = cload(shc, [128, 1], F32, "shc")
            mxr_sb = cload(mxr, [128, 1], F32, "mxr")
            idb_sb = cload(idb, [128, 128], BF16, "idb")

            rv = cpool.tile([3, 1], F32, tag="rv")
            rs = cpool.tile([3, 1], F32, tag="rs")
            nc.vector.reciprocal(rv[:], vr_sb[:])
            nc.vector.reciprocal(rs[:], ss_sb[:])
            rvr_sb = cpool.tile([128, 1], F32, tag="rvr")
            soff_sb = cpool.tile([128, 1], F32, tag="soff")
            with tc.tile_pool(name="psc", bufs=1, space="PSUM") as pscp:
                ps_sc = pscp.tile([128, 2], F32, tag="psc")
                nc.tensor.matmul(ps_sc[:, 0:1], selvr_sb[:], rv[:],
                                 start=True, stop=True)
                nc.tensor.matmul(ps_sc[:, 1:2], selss_sb[:], rs[:],
                                 start=True, stop=True)
                nc.vector.tensor_copy(rvr_sb[:], ps_sc[:, 0:1])
                nc.vector.tensor_copy(soff_sb[:], ps_sc[:, 1:2])

            tables = dpool.tile([HM, LEN, 256], BF16, tag="tables")
            xtb = dpool.tile([2, 128, LEN], BF16, tag="xtb")

            for _it in range(niter):
                _body(nc, tc, locals())

    nc.compile()
    return nc


def _body(nc, tc, env):
    (xt, grid, wv_sb, wp_sb, wo_sb, bval_sb, bproj_sb, selsum_sb, selrep_sb,
     shr_sb, shc_sb, mxr_sb, pfold_sb, idb_sb, rvr_sb, soff_sb, tables, xtb,
     outp, fpool) = (
        env[k] for k in (
            "xt", "grid", "wv_sb", "wp_sb", "wo_sb", "bval_sb", "bproj_sb",
            "selsum_sb", "selrep_sb", "shr_sb", "shc_sb", "mxr_sb",
            "pfold_sb", "idb_sb", "rvr_sb", "soff_sb", "tables", "xtb",
            "outp", "fpool"))
    if True:
        if True:
            # ========== PHASE B: value, G volumes, tables ==========
            with (
                tc.tile_pool(name="gvol", bufs=1) as gpool,
                tc.tile_pool(name="stageB", bufs=2) as spool,
                tc.tile_pool(name="workB", bufs=2) as wpool,
                tc.tile_pool(name="psB", bufs=2, space="PSUM") as psb,
            ):
                for rc in range(4):
                    c0 = rc * RCH
                    cend = min(c0 + RCH + 1024, LEN)
                    ncols = cend - c0
                    vt = gpool.tile([128, RCH + 1024], BF16, tag="vt")
                    for s in range((ncols + 511) // 512):
                        a = c0 + s * 512
                        w = min(512, cend - a)
                        xq = wpool.tile([128, 2, 512], F32, tag="xq")
                        nc.sync.dma_start(out=xq[:, 0, :w], in_=xt[0, :, a:a + w])
                        nc.sync.dma_start(out=xq[:, 1, :w], in_=xt[1, :, a:a + w])
                        xqb = wpool.tile([128, 2, 512], BF16, tag="xqb")
                        nc.vector.tensor_copy(xqb[:, :, :w], xq[:, :, :w])
                        if s < 8:  # halo cols are written by the next chunk
                            nc.sync.dma_start(out=xtb[0, :, a:a + w],
                                              in_=xqb[:, 0, :w])
                            nc.sync.dma_start(out=xtb[1, :, a:a + w],
                                              in_=xqb[:, 1, :w])
                        pv = psb.tile([128, 512], F32, tag="pv")
                        nc.tensor.matmul(pv[:, :w], wv_sb[:, 0, :],
                                         xqb[:, 0, :w], start=True, stop=False)
                        nc.tensor.matmul(pv[:, :w], wv_sb[:, 1, :],
                                         xqb[:, 1, :w], start=False, stop=True)
                        nc.vector.tensor_scalar(vt[:, s * 512:s * 512 + w],
                                                pv[:, :w], bval_sb[:], None,
                                                OP.add)

                    gx = gpool.tile([128, RCH], BF16, tag="gx")
                    gy = gpool.tile([128, RCH], BF16, tag="gy")
                    gxy = gpool.tile([128, RCH], BF16, tag="gxy")
                    gd = gpool.tile([128, RCH], BF16, tag="gd")
                    gdx = gpool.tile([128, RCH], BF16, tag="gdx")
                    gdy = gpool.tile([128, RCH], BF16, tag="gdy")
                    gdxy = gpool.tile([128, RCH], BF16, tag="gdxy")

                    def dshift_x(dst, src):
                        s3 = src[:, 0:RCH].rearrange("p (r x) -> p r x", x=32)
                        d3 = dst[:, 0:RCH].rearrange("p (r x) -> p r x", x=32)
                        nc.vector.tensor_tensor(d3[:, :, 0:31], s3[:, :, 1:32],
                                                s3[:, :, 0:31], OP.subtract)
                        nc.vector.tensor_scalar(d3[:, :, 31:32], s3[:, :, 31:32],
                                                -1.0, None, OP.mult)

                    def dshift_y(dst, src):
                        s4 = src[:, 0:RCH].rearrange(
                            "p (d y x) -> p d y x", y=32, x=32)
                        d4 = dst[:, 0:RCH].rearrange(
                            "p (d y x) -> p d y x", y=32, x=32)
                        nc.vector.tensor_tensor(d4[:, :, 0:31, :],
                                                s4[:, :, 1:32, :],
                                                s4[:, :, 0:31, :], OP.subtract)
                        nc.vector.tensor_scalar(d4[:, :, 31:32, :],
                                                s4[:, :, 31:32, :],
                                                -1.0, None, OP.mult)

                    def dshift_d(dst, src_full):
                        lim = min(RCH, 15 * 1024 - c0)
                        if lim > 0:
                            nc.vector.tensor_tensor(
                                dst[:, 0:lim], src_full[:, 1024:1024 + lim],
                                src_full[:, 0:lim], OP.subtract)
                        if lim < RCH:
                            nc.vector.tensor_scalar(
                                dst[:, lim:RCH], src_full[:, lim:RCH],
                                -1.0, None, OP.mult)

                    dshift_x(gx, vt)
                    dshift_y(gy, vt)
                    dshift_x(gxy, gy)
                    dshift_d(gd, vt)
                    dshift_x(gdx, gd)
                    dshift_y(gdy, gd)
                    dshift_x(gdxy, gdy)
                    gvols = [vt, gx, gy, gxy, gd, gdx, gdy, gdxy]

                    for grp in range(4):
                        stg = spool.tile([128, 8, HM, DH, 8], BF16, tag="stg")
                        for sub in range(8):
                            pt = psb.tile([128, 1024], BF16, tag="pt")
                            off = (grp * 8 + sub) * 128
                            for g in range(8):
                                nc.tensor.transpose(
                                    pt[:, g * 128:(g + 1) * 128],
                                    gvols[g][:, off:off + 128], idb_sb[:])
                            pt4 = pt.rearrange("p (g h c) -> p g h c",
                                               g=8, h=HM)
                            for gh in range(2):
                                src = pt4[:, gh * 4:(gh + 1) * 4, :, :]
                                dst = stg[:, sub, :, :, gh * 4:(gh + 1) * 4]
                                dstv = dst.rearrange("p h c g -> p g h c")
                                if gh == 0:
                                    nc.vector.tensor_copy(dstv, src)
                                else:
                                    nc.scalar.copy(dstv, src)
                        r_base = c0 + grp * 1024
                        for h in range(HM):
                            tdst = tables[h, r_base:r_base + 1024, :].rearrange(
                                "(s r) cg -> r s cg", s=8)
                            nc.sync.dma_start(
                                out=tdst,
                                in_=stg[:, :, h, :, :].rearrange(
                                    "r s c g -> r s (c g)"))

            # ========== PHASE C: coords, coefs, attn ==========
            coefa = fpool.tile([128, 64, 2, 4, HM, P], BF16, tag="coefa")
            coefb = fpool.tile([128, 64, 2, 4, HM, P], BF16, tag="coefb")
            coefc = fpool.tile([128, 64, 2, 4, HM, P], BF16, tag="coefc")
            with (
                tc.tile_pool(name="tin", bufs=1) as tpool,
                tc.tile_pool(name="workC", bufs=1) as wpc,
                tc.tile_pool(name="psC", bufs=2, space="PSUM") as psc,
            ):
                tin_a = tpool.tile([128, QH], BF16, tag="tin_a")
                tin_b = tpool.tile([128, QH], BF16, tag="tin_b")
                tin_c = tpool.tile([128, QH], BF16, tag="tin_c")
                nc.vector.memset(tin_c[:, :], 0.0)
                nc.vector.memset(tin_a[:, :], 0.0)
                for b in range(NBLK):
                    q0 = b * BLK
                    xq0 = wpc.tile([128, 2, BLK], BF16, tag="cxq0")
                    xq1 = wpc.tile([128, 2, BLK], BF16, tag="cxq1")
                    for k in range(2):
                        nc.sync.dma_start(out=xq0[:, k, :],
                                          in_=xtb[k, :, q0:q0 + BLK])
                        nc.sync.dma_start(out=xq1[:, k, :],
                                          in_=xtb[k, :, QH + q0:QH + q0 + BLK])
                    pj = wpc.tile([128, BLK], F32, tag="pj")
                    for s in range(BLK // 512):
                        pp = psc.tile([128, 512], F32, tag="pp")
                        for qh, xqh in ((0, xq0), (1, xq1)):
                            sl = slice(qh * 64, qh * 64 + 64)
                            nc.tensor.matmul(
                                pp[sl, :], wp_sb[:, 0, :],
                                xqh[:, 0, s * 512:(s + 1) * 512],
                                start=True, stop=False)
                            nc.tensor.matmul(
                                pp[sl, :], wp_sb[:, 1, :],
                                xqh[:, 1, s * 512:(s + 1) * 512],
                                start=False, stop=True)
                        nc.vector.tensor_copy(pj[:, s * 512:(s + 1) * 512],
                                              pp[:])

                    gr = wpc.tile([128, BLK], F32, tag="gr")
                    nc.sync.dma_start(out=gr[:], in_=grid[:, q0:q0 + BLK])

                    pg = wpc.tile([128, BLK], F32, tag="pg")
                    z = wpc.tile([128, BLK], F32, tag="z")
                    i32t = wpc.tile([128, BLK], I32, tag="i32")
                    zf = wpc.tile([128, BLK], F32, tag="zf")
                    t0 = wpc.tile([128, BLK], F32, tag="t0")
                    t1 = wpc.tile([128, BLK], F32, tag="t1")
                    frac = wpc.tile([128, BLK], F32, tag="frac")
                    f0 = wpc.tile([128, BLK], F32, tag="f0")

                    bsl = slice(q0, q0 + BLK)
                    nc.vector.tensor_scalar(pg[:], gr[:], rvr_sb[:], shc_sb[:],
                                            OP.mult, OP.add)
                    nc.vector.tensor_scalar(t0[:], pj[:], bproj_sb[:],
                                            soff_sb[:], OP.add, OP.mult)
                    nc.vector.tensor_tensor(z[:], t0[:], pg[:], OP.add)
                    nc.vector.tensor_copy(i32t[:], z[:])
                    nc.vector.tensor_copy(zf[:], i32t[:])
                    nc.vector.tensor_tensor(t0[:], zf[:], z[:], OP.is_gt)
                    nc.vector.tensor_tensor(zf[:], zf[:], t0[:], OP.subtract)
                    nc.vector.tensor_tensor(frac[:], z[:], zf[:], OP.subtract)
                    nc.vector.tensor_scalar(f0[:], zf[:], shr_sb[:], None,
                                            OP.subtract)
                    # ---- attention first (widened to 32-aligned windows,
                    # garbage rows are overwritten by the coef writes below)
                    nc.scalar.activation(tin_a[32:64, bsl], pj[32:64, :],
                                         ACTF.Exp, bias=bproj_sb[32:64, :])
                    nc.scalar.activation(tin_a[96:128, bsl], pj[96:128, :],
                                         ACTF.Exp, bias=bproj_sb[96:128, :])
                    rc8 = wpc.tile([8, BLK], F32, tag="rc8")
                    for s in range(BLK // 512):
                        ssl = slice(s * 512, (s + 1) * 512)
                        qsl = slice(q0 + s * 512, q0 + (s + 1) * 512)
                        pr = psc.tile([64, 512], F32, tag="pr")
                        nc.tensor.matmul(pr[32:40, :], selsum_sb[:],
                                         tin_a[:, qsl], start=True, stop=True)
                        nc.vector.reciprocal(rc8[:, ssl], pr[32:40, :])
                        nc.tensor.matmul(pr[0:64, :], selrep_sb[:],
                                         rc8[:, ssl], start=True, stop=True)
                        nc.vector.tensor_tensor(tin_a[32:64, qsl],
                                                tin_a[32:64, qsl],
                                                pr[0:32, :], OP.mult)
                        nc.vector.tensor_tensor(tin_a[96:128, qsl],
                                                tin_a[96:128, qsl],
                                                pr[32:64, :], OP.mult)
                    # ---- per-axis coefficient writes (overwrite garbage)
                    nc.vector.tensor_scalar(tin_c[0:48, bsl], f0[0:48, :],
                                            0.0, mxr_sb[0:48, :],
                                            OP.max, OP.min)
                    nc.vector.tensor_scalar(tin_c[64:112, bsl], f0[64:112, :],
                                            0.0, mxr_sb[64:112, :],
                                            OP.max, OP.min)
                    nc.vector.tensor_scalar(t0[:], f0[:], 0.0, None, OP.is_ge)
                    nc.vector.tensor_scalar(t1[:], f0[:], mxr_sb[:], None,
                                            OP.is_le)
                    nc.vector.tensor_tensor(t0[:], t0[:], t1[:], OP.mult)
                    nc.vector.tensor_scalar(t1[:], f0[:], -1.0, None,
                                            OP.is_equal)
                    nc.vector.tensor_tensor(t1[:], t1[:], frac[:], OP.mult)
                    nc.vector.tensor_tensor(tin_b[:, bsl], t0[:], frac[:],
                                            OP.mult)
                    nc.vector.tensor_tensor(tin_a[0:48, bsl], t0[0:48, :],
                                            t1[0:48, :], OP.add)
                    nc.vector.tensor_tensor(tin_a[64:112, bsl], t0[64:112, :],
                                            t1[64:112, :], OP.add)

                # ---- transpose TIN -> s-layout coefs ----
                for tin, coef, eng in ((tin_a, coefa, 0), (tin_b, coefb, 1),
                                       (tin_c, coefc, 0)):
                    cflat = coef.rearrange("p a b c d e -> p (a b c d e)")
                    for cb in range(16):
                        pt = psc.tile([128, 512], BF16, tag="ptt")
                        for j in range(4):
                            qol = cb * 4 + j
                            nc.tensor.transpose(
                                pt[:, j * 128:(j + 1) * 128],
                                tin[:, qol * 128:(qol + 1) * 128], idb_sb[:])
                        if eng == 0:
                            nc.vector.tensor_copy(
                                cflat[:, cb * 512:(cb + 1) * 512], pt[:])
                        else:
                            nc.scalar.copy(
                                cflat[:, cb * 512:(cb + 1) * 512], pt[:])

            # prefold attn into the d-axis pair (grp0 *= grp3)
            def gview(coef, g):
                return coef.rearrange(
                    "p ql qh g h pp -> p (ql qh) g (h pp)")[:, :, g, :]

            nc.vector.tensor_tensor(gview(coefa, 0), gview(coefa, 0),
                                    gview(coefa, 3), OP.mult)
            nc.vector.tensor_tensor(gview(coefb, 0), gview(coefb, 0),
                                    gview(coefa, 3), OP.mult)

            # ========== PHASES D/E: gather + weighted reduce ==========
            sampled = fpool.tile([128, 64, 2, HM, DH], BF16, tag="sampled")
            with (
                tc.tile_pool(name="gath", bufs=2) as hpool,
                tc.tile_pool(name="psE", bufs=2, space="PSUM") as pse,
            ):
                for h in range(HM):
                    # per-head coef views [128, (ql qh), pp] for grp g
                    def hview(coef, g):
                        return coef[:, :, :, g, h, :].rearrange(
                            "p ql qh pp -> p (ql qh) pp")

                    c8 = hpool.tile([128, NSLOT, 8], BF16, tag="c8")
                    c8v = c8.rearrange("p (s pp) g -> p s pp g", pp=P)
                    for bc in range(4):
                        yv = hview(coefa, 2) if bc < 2 else hview(coefb, 2)
                        xv = hview(coefa, 1) if bc % 2 == 0 else hview(coefb, 1)
                        nc.vector.tensor_tensor(c8v[:, :, :, bc], yv, xv,
                                                OP.mult)
                    for bc in range(4):
                        nc.vector.tensor_tensor(c8v[:, :, :, 4 + bc],
                                                c8v[:, :, :, bc],
                                                hview(coefb, 0), OP.mult)
                    for bc in range(4):
                        nc.vector.tensor_tensor(c8v[:, :, :, bc],
                                                c8v[:, :, :, bc],
                                                hview(coefa, 0), OP.mult)

                    r0h = hpool.tile([128, NSLOT], F32, tag="r0h")
                    r0v = r0h.rearrange("p (s pp) -> p s pp", pp=P)
                    nc.vector.tensor_scalar(r0v[:], hview(coefc, 0), 1024.0,
                                            None, OP.mult)
                    nc.vector.scalar_tensor_tensor(r0v[:], hview(coefc, 2),
                                                   32.0, r0v[:],
                                                   OP.mult, OP.add)
                    nc.vector.scalar_tensor_tensor(r0v[:], hview(coefc, 1),
                                                   1.0, r0v[:],
                                                   OP.mult, OP.add)

                    idxw = hpool.tile([128, NSLOT * 8], I16, tag="idxw")
                    for g in range(8):
                        pf = pse.tile([16, NSLOT], F32, tag="pf")
                        nc.tensor.matmul(pf[:], pfold_sb[:, g, :],
                                         r0h[:], start=True, stop=True)
                        iv = idxw[0:16, :].rearrange("p (s g) -> p s g", g=8)
                        nc.vector.tensor_copy(iv[:, :, g], pf[:])
                    for rep in range(1, 8):
                        nc.sync.dma_start(out=idxw[rep * 16:(rep + 1) * 16, :],
                                          in_=idxw[0:16, :])

                    for call in range(NCALL):
                        gb = hpool.tile([128, GSLOT, 256], BF16, tag="gb")
                        nc.gpsimd.dma_gather(
                            gb[:], tables[h],
                            idxw[:, call * 256:(call + 1) * 256],
                            GCALL, GCALL, 256, single_packet=False)
                        s0 = call * GSLOT
                        for hf in range(2):
                            sl = slice(s0 + hf * 16, s0 + hf * 16 + 16)
                            gsl = slice(hf * 16, hf * 16 + 16)
                            tt = hpool.tile([128, 16, DH, 8], BF16, tag="tt")
                            gv = gb[:, gsl, :].rearrange(
                                "p s (c g) -> p s c g", g=8)
                            cv = c8[:, sl, :].unsqueeze(2).broadcast_to(
                                (128, 16, DH, 8))
                            nc.vector.tensor_tensor(tt[:], gv, cv, OP.mult)
                            rr = hpool.tile([128, 16, DH], F32, tag="rr")
                            nc.vector.tensor_reduce(rr[:], tt[:], AX.X, OP.add)
                            r4 = rr.rearrange(
                                "p (s pp) c -> p s pp c", pp=P)
                            a0 = hpool.tile([128, 4, DH], F32, tag="a0")
                            nc.vector.tensor_tensor(a0[:], r4[:, :, 0],
                                                    r4[:, :, 1], OP.add)
                            nc.vector.tensor_tensor(r4[:, :, 0], r4[:, :, 2],
                                                    r4[:, :, 3], OP.add)
                            qlb = (call * 4 + hf * 2)
                            sview = sampled[:, qlb:qlb + 2, :, h, :].rearrange(
                                "p ql qh c -> p (ql qh) c")
                            nc.vector.tensor_tensor(
                                sview, a0[:], r4[:, :, 0], OP.add)

            # ========== PHASE F: out projection ==========
            with (
                tc.tile_pool(name="workF", bufs=2) as wpf,
                tc.tile_pool(name="psF", bufs=2, space="PSUM") as psf,
            ):
                for qh in range(2):
                    for ob in range(8):
                        ot = wpf.tile([128, 8, 256], F32, tag="ot")
                        for j in range(8):
                            qol = ob * 8 + j
                            ptx = psf.tile([128, 128], BF16, tag="ptx")
                            sv = sampled[:, qol, qh, :, :].rearrange(
                                "p h c -> p (h c)")
                            nc.tensor.transpose(ptx[:], sv, idb_sb[:])
                            lt = wpf.tile([128, 128], BF16, tag="lt")
                            nc.vector.tensor_copy(lt[:], ptx[:])
                            po = psf.tile([128, 256], F32, tag="po")
                            nc.tensor.matmul(po[:], lt[:], wo_sb[:],
                                             start=True, stop=True)
                            nc.scalar.copy(ot[:, j, :], po[:])
                        q0 = qh * QH + ob * 1024
                        dst = outp[q0:q0 + 1024, :].rearrange(
                            "(j qi) c -> qi j c", j=8)
                        nc.sync.dma_start(out=dst, in_=ot[:])


# ---------------- host side ----------------

_prog_cache = {}


def _get_prog():
    if "nc" not in _prog_cache:
        _prog_cache["nc"] = build_program()
    return _prog_cache["nc"]


def _host_consts():
    c = {}
    q = np.arange(LEN)
    rows = np.zeros((128, LEN), np.float32)
    gfun = {0: lambda qq: 0.5 + qq // 1024, 1: lambda qq: 0.5 + qq % 32,
            2: lambda qq: 0.5 + (qq // 32) % 32}
    # grid is indexed [row, q%QH] per q-half block; row encodes qh
    half = np.zeros((128, QH), np.float32)
    for qh in range(2):
        qq = np.arange(QH) + qh * QH
        for ax in range(3):
            half[qh * 64 + ax * 16:qh * 64 + ax * 16 + 16, :] = \
                gfun[ax](qq)[None, :]
    c["grid"] = np.ascontiguousarray(half)

    selvr = np.zeros((3, 128), np.float32)
    selss = np.zeros((3, 128), np.float32)
    shr = np.zeros((128, 1), np.float32)
    mxr = np.ones((128, 1), np.float32)
    vr_idx = {0: 0, 1: 1, 2: 2}
    ss_idx = {0: 0, 1: 2, 2: 1}
    ss_scl = {0: float(D_), 1: float(W_), 2: float(H_)}
    sh_v = {0: 16.0, 1: 32.0, 2: 32.0}
    mx_v = {0: 15.0, 1: 31.0, 2: 31.0}
    for qh in range(2):
        for ax in range(3):
            sl = slice(qh * 64 + ax * 16, qh * 64 + ax * 16 + 16)
            selvr[vr_idx[ax], sl] = 1.0
            selss[ss_idx[ax], sl] = ss_scl[ax]
            shr[sl] = sh_v[ax]
            mxr[sl] = mx_v[ax]
    c["selvr"], c["selss"], c["shr"], c["mxr"] = selvr, selss, shr, mxr
    c["shc"] = shr - 0.5

    selsum = np.zeros((128, 8), np.float32)
    for qh in range(2):
        for m in range(4):
            selsum[qh * 64 + 48 + m * 4:qh * 64 + 48 + (m + 1) * 4,
                   qh * 4 + m] = 1.0
    c["selsum"] = selsum

    selrep = np.zeros((8, 64), np.float32)
    for qh in range(2):
        for m in range(4):
            base = 16 + qh * 32
            selrep[qh * 4 + m, base + m * 4:base + (m + 1) * 4] = 1.0
    c["selrep"] = selrep

    pfold = np.zeros((128, 8, 16), np.float32)
    for g in range(8):
        for r in range(16):
            pfold[g * 16 + r, g, r] = 1.0
    c["pfold"] = pfold

    c["idb"] = np.eye(128, dtype=np.float32)
    return c


def _build_in_maps(inputs):
    import ml_dtypes
    bf = ml_dtypes.bfloat16

    x = np.asarray(inputs["x"], np.float32)
    vr = np.asarray(inputs["valid_ratios"], np.float32)
    Wv = np.asarray(inputs["W_value"], np.float32)
    bv = np.asarray(inputs["b_value"], np.float32)
    Wo = np.asarray(inputs["W_off"], np.float32)
    bo = np.asarray(inputs["b_off"], np.float32)
    Wa = np.asarray(inputs["W_attn"], np.float32)
    ba = np.asarray(inputs["b_attn"], np.float32)
    Wu = np.asarray(inputs["W_out"], np.float32)
    bu = np.asarray(inputs["b_out"], np.float32)
    ss = np.asarray(inputs["input_spatial_shapes"]).astype(np.float32)

    consts = _host_consts()

    in_maps = []
    for core in range(8):
        n, hg = core // 2, core % 2
        m = {}
        m["xt"] = np.ascontiguousarray(x[n].T).reshape(2, 128, LEN)
        m["grid"] = consts["grid"]
        m["wv"] = np.ascontiguousarray(
            Wv[:, hg * 128:(hg + 1) * 128].astype(bf)).reshape(2, 128, 128)
        Wof = Wo.reshape(DIM, M_TOT, P, 3)[:, hg * 4:(hg + 1) * 4]
        wp = np.zeros((DIM, 64), np.float32)
        for ax in range(3):
            wp[:, ax * 16:(ax + 1) * 16] = Wof[:, :, :, ax].reshape(DIM, 16)
        wp[:, 48:64] = Wa[:, hg * 16:(hg + 1) * 16]
        m["wproj"] = np.ascontiguousarray(wp.astype(bf)).reshape(2, 128, 64)
        m["wout"] = np.ascontiguousarray(
            Wu[hg * 128:(hg + 1) * 128, :].astype(bf))
        m["bval"] = np.ascontiguousarray(
            bv[hg * 128:(hg + 1) * 128].reshape(128, 1))
        bof = bo.reshape(M_TOT, P, 3)[hg * 4:(hg + 1) * 4]
        bp = np.zeros((128, 1), np.float32)
        for qh in range(2):
            for ax in range(3):
                bp[qh * 64 + ax * 16:qh * 64 + (ax + 1) * 16, 0] = \
                    bof[:, :, ax].reshape(16)
            bp[qh * 64 + 48:qh * 64 + 64, 0] = ba[hg * 16:(hg + 1) * 16]
        m["bproj"] = bp
        for k in ("selvr", "selss", "shr", "shc", "mxr", "selrep", "pfold",
                  "grid"):
            m[k] = consts[k]
        m["selsum"] = consts["selsum"].astype(bf)
        m["idb"] = consts["idb"].astype(bf)
        m["vr3"] = np.ascontiguousarray(vr[n].reshape(3, 1))
        m["ss3"] = np.ascontiguousarray(ss.reshape(3, 1))
        in_maps.append(m)
    return in_maps, bu


def kernel(**inputs):
    in_maps, bu = _build_in_maps(inputs)
    nc = _get_prog()
    res = run_bass_kernel_spmd(nc, in_maps, list(range(8))).results
    out = np.zeros((4, LEN, DIM), np.float32)
    for n in range(4):
        out[n] = res[2 * n]["outp"] + res[2 * n + 1]["outp"] + bu[None, :]
    return out
